# revision 48
# baseline (speedup 1.0000x reference)
"""Trainium2 Bass kernel for Mistral-style sliding-window GQA attention.

Problem (hardcoded shapes):
  hidden_states [2048, 4096] f32, Wq [4096, 4096], Wk/Wv [4096, 1024],
  Wo [4096, 4096], cu_seqlens [3] int32. 32 Q heads / 8 KV heads,
  head_dim 128, sliding window 512, rope theta 10000.

Sharding: tensor-parallel over heads across 8 cores. Core c owns Q heads
[4c, 4c+4) and KV head c (GQA groups align: qh//4 == c). Wq/Wk/Wv are
column-sharded, Wo row-sharded; each core emits a partial [2048, 4096]
output which the host sums.

Device kernel layout choices (per core):
  - The four big GEMMs (q/k/v projections, out-proj) run in fp8 e4m3
    DoubleRow mode (2 k-tiles per PE instruction at 0.5 cycles/row) with
    a 3-term residual decomposition X@W ~= X8@W8 + RX8@W8 + X8@RW8.
    Operands are quantized at power-of-2 scales (X*32, W*1024, A*16)
    with residuals on the same grid, so all three terms accumulate in
    one PSUM bank and the single descale folds into existing table /
    copy steps (cos/sin tables, v copy, out-proj PSUM drain). Each bank
    fill has exactly ONE start=True (first instr) and ONE stop=True
    (last instr): start marks the whole 2KB bank pending-zero, so each
    column-half's first write lands on zeroed bytes.
  - hT = hidden^T in e4m3 (x8 ++ rx8 per 4-ktile group, one DMA) is the
    streamed rhs for all projections; weight tiles (w8 + rw8) are the
    stationary operand.
  - RoPE: rotate_half is two partition-shifted DVE multiplies against a
    sign-folded sin table; no PE work. Tables carry the 2^-15 descale.
  - scores are computed transposed (ST[k,q] = kT.T @ qT) for two heads
    at once in bf16; softmax skips max-subtraction; the denominator
    comes free as a ones-column appended to V.
  - partial-tile masks are host-computed 0/1 bf16 tiles applied
    multiplicatively after exp on the DVE.
  - attention output (x16) is normalized per-partition, transposed on
    the PE in bf16, then DVE-quantized to e4m3 + residual head-pair
    tiles feeding the fp8 out-proj; partials bounce PSUM->SBUF with a
    2^-14 descale and stream to DRAM as bf16 (host sums 8 partials).
  - scheduling: per-group weight DMAs (one 6KB-row transfer) keep strip
    0 DMA-bound work off the PE critical path; the first two tiles'
    score blocks are hoisted into phase 1 to cover the pool-boundary
    drain; phase 2 emits score blocks ahead of PV blocks and interleaves
    the lagged out-proj so the in-order PE queue never sits behind an
    exp; token tile 0 (single job) is processed last to minimize the
    final serial chain.
"""

import numpy as np
import ml_dtypes

import concourse.bass as bass
import concourse.tile as tile
from concourse import bacc, mybir
from concourse import bass_utils

# ---- problem constants -------------------------------------------------
T = 2048
HID = 4096
NUM_HEADS = 32
NUM_KV_HEADS = 8
D = 128  # head dim
WINDOW = 512
ROPE_THETA = 10000.0
N_CORES = 8
HPC = NUM_HEADS // N_CORES  # 4 q heads per core
QD = HPC * D  # 512 q-proj cols per core

NT = T // 128  # 16 token tiles
NKT = HID // 128  # 32 hidden k-tiles
NSTRIP = T // 512  # 4 token strips of 512
NOUT = HID // 512  # 8 output column slices

F32 = mybir.dt.float32
BF16 = mybir.dt.bfloat16
E4 = mybir.dt.float8e4
E4NP = ml_dtypes.float8_e4m3
SCALE = 1.0 / np.sqrt(D)
DR = mybir.MatmulPerfMode.DoubleRow

# fp8 quantization scales (powers of 2; residuals on the same grid)
SX = 32.0       # hidden_states
SW = 1024.0     # Wq/Wk/Wv
SA = 16.0       # attention output (folded into v descale)
SWO = 1024.0    # Wo
DESCALE_QK = 1.0 / (SX * SW)          # folded into cos/sin tables
DESCALE_V = SA / (SX * SW)            # applied at v PSUM->SBUF copy
DESCALE_O = 1.0 / (SA * SWO)          # applied at out-proj PSUM drain

_cache = {}


def _q8(x):
    """e4m3 quantize + same-grid residual (both as e4m3 arrays)."""
    b = x.astype(E4NP)
    r = (x - b.astype(np.float32)).astype(E4NP)
    return b, r


def _host_prep(cu_seqlens):
    """Everything derived from cu_seqlens: positions, rope tables,
    per-tile job list and mask tiles (ST layout [k, q], head-pair
    duplicated to [128, 256])."""
    cu = np.asarray(cu_seqlens, dtype=np.int64)
    tok = np.arange(T)
    seg = np.searchsorted(cu[1:], tok, side="right")
    pos = tok - cu[np.minimum(seg, len(cu) - 1)]

    same = seg[:, None] == seg[None, :]
    causal = pos[None, :] <= pos[:, None]
    win = pos[None, :] >= pos[:, None] - (WINDOW - 1)
    allowed = same & causal & win  # [q, k]

    jobs = []  # jobs[i] = [(j, mask_id | None), ...]
    masks = []
    mask_index = {}
    for i in range(NT):
        row = []
        for j in range(NT):
            blk = allowed[128 * i : 128 * (i + 1), 128 * j : 128 * (j + 1)]
            if not blk.any():
                continue
            if blk.all():
                row.append((j, None))
            else:
                key = blk.tobytes()
                if key not in mask_index:
                    mask_index[key] = len(masks)
                    masks.append(blk.T.astype(np.float32))  # ST layout
                row.append((j, mask_index[key]))
        jobs.append(row)
    if not masks:
        masks.append(np.ones((128, 128), np.float32))
    m = np.stack(masks)
    m = np.concatenate([m, m], axis=2).astype(ml_dtypes.bfloat16)
    # single [128, n_masks*256] tensor for one batched DMA
    masks_np = np.ascontiguousarray(
        m.transpose(1, 0, 2).reshape(128, -1))

    inv = 1.0 / (ROPE_THETA ** (np.arange(0, D, 2, dtype=np.float64) / D))
    freqs = pos[:, None].astype(np.float64) * inv[None, :]  # [T, 64]
    emb = np.concatenate([freqs, freqs], axis=1)  # [T, 128]
    # tables carry the fp8 descale for the q/k projections
    cos_t = (np.cos(emb).T * DESCALE_QK).astype(np.float32).copy()
    sin_t = (np.sin(emb).T * DESCALE_QK).astype(np.float32)
    # sign-folded: rope(x)[d] = x[d]*cos[d] + x[(d+64)%128] * sin_s[d]
    sin_s = np.concatenate([-sin_t[:64], sin_t[64:]], axis=0).copy()
    ident = np.eye(128, dtype=ml_dtypes.bfloat16)

    return jobs, masks_np, cos_t, sin_s, ident


def _build(jobs, n_masks):
    """Trace the Bass/Tile program (identical on all cores)."""
    nc = bacc.Bacc("TRN2", target_bir_lowering=False, debug=False,
                   num_devices=N_CORES)

    # DRAM I/O (per-core shapes). ht carries x8 then rx8 for each
    # 4-ktile group: [strip, group, 128, base/resid, ktile, token]
    ht_d = nc.dram_tensor("ht", [NSTRIP, NKT // 4, 128, 2, 4, 512], E4,
                          kind="ExternalInput").ap()
    # all 12 qkv weight tensors (4 wq + wk + wv, base then resid),
    # group-interleaved so strip 0 loads one 6KB-row DMA per k-group:
    # [group, 128, tensor, kpair-in-group, sub, 128]
    wall_d = nc.dram_tensor("wall", [NKT // 4, 128, 12, 2, 2, 128], E4,
                            kind="ExternalInput").ap()
    # wo: [base/resid, head-pair, d, head-in-pair, outcol]
    wo_d = nc.dram_tensor("wo", [2, 2, 128, 2, HID], E4,
                          kind="ExternalInput").ap()
    cos_d = nc.dram_tensor("cos_t", [128, T], F32, kind="ExternalInput").ap()
    sin_d = nc.dram_tensor("sin_s", [128, T], F32, kind="ExternalInput").ap()
    ident_d = nc.dram_tensor("ident", [128, 128], BF16,
                             kind="ExternalInput").ap()
    masks_d = nc.dram_tensor("masks", [128, n_masks * 256], BF16,
                             kind="ExternalInput").ap()
    out_d = nc.dram_tensor("out", [T, HID], BF16, kind="ExternalOutput").ap()

    with tile.TileContext(nc) as tc:
        with tc.tile_pool(name="persist", bufs=1) as pp:
            # resident weights / tables (base + residual, e4m3)
            # w_all: [128, group, tensor(12), kpair-in-group, sub, 128];
            # tensor order: wq base h0-3, wk base, wv base, then residuals
            w_all = pp.tile([128, NKT // 4, 12, 2, 2, 128], E4, name="w_all")
            wo_sb = [[pp.tile([128, 2, HID], E4, name=f"wo{r}_{hp}")
                      for hp in range(2)] for r in range(2)]
            cos_sb = pp.tile([128, T], F32, name="cos_sb")
            sin_sb = pp.tile([128, T], F32, name="sin_sb")
            ident_sb = pp.tile([128, 128], BF16, name="ident_sb")
            mask_sb = pp.tile([128, n_masks * 256], BF16, name="mask_sb")
            # activations produced by phase 1, consumed by phase 2.
            # Per-strip tiles so phase-2 reads only depend on their strip's
            # rope. qt strip tiles: cols [256*i + 128*m : +128] = head
            # (2*hp + m), token tile 4s+i.
            qts = [[pp.tile([128, 1024], BF16, name=f"qtp{hp}_{s}")
                    for s in range(NSTRIP)] for hp in range(2)]
            kts = [pp.tile([128, 512], BF16, name=f"kt{s}")
                   for s in range(NSTRIP)]
            vaug_sb = [pp.tile([128, D + 1], BF16, name=f"vaug{t}")
                       for t in range(NT)]

            qt_4d = [[q.rearrange("p (i m c) -> p i m c", m=2, c=128)
                      for q in qs] for qs in qts]

            for t in range(NT):
                nc.vector.memset(vaug_sb[t][:, D : D + 1], 1.0)

            def score_block_gen(i, hp, ps_alloc, se_alloc):
                """Scores + exp + mask for all job pairs of (i, hp).
                Returns the se tiles (one per pair)."""
                jl = jobs[i]
                ses = []
                for p0 in range(0, len(jl), 2):
                    pair = jl[p0 : p0 + 2]
                    ps_s = ps_alloc(f"pss{i}_{hp}_{p0}")
                    for q, (j, mid) in enumerate(pair):
                        nc.tensor.matmul(
                            ps_s[:, bass.ts(q, 256)],
                            kts[j // 4][:, bass.ts(j % 4, 128)],
                            qts[hp][i // 4][:, bass.ts(i % 4, 256)],
                            start=True, stop=True)
                    se = se_alloc(f"se{i}_{hp}_{p0}")
                    # exp per job: the first PV matmul only waits on a
                    # 256-wide activation instead of the full pair
                    for q, (j, mid) in enumerate(pair):
                        qsl = bass.ts(q, 256)
                        nc.scalar.activation(
                            se[:, qsl], ps_s[:, qsl],
                            mybir.ActivationFunctionType.Exp,
                            bias=0.0, scale=float(SCALE))
                        if mid is not None:
                            nc.vector.tensor_mul(
                                se[:, qsl], se[:, qsl],
                                mask_sb[:, bass.ts(mid, 256)])
                    ses.append(se)
                return ses

            early_ses = {}

            # ---------------- phase 1: projections + RoPE ----------------
            with (
                tc.tile_pool(name="ht_pool", bufs=6) as htp,
                tc.tile_pool(name="rope_tmp", bufs=4) as rtp,
                tc.tile_pool(name="proj_psum", bufs=6, space="PSUM") as ppp,
                tc.tile_pool(name="util_psum", bufs=2, space="PSUM") as upp,
            ):
                def rope(s, h, src):
                    """src: fp32 PSUM [128, 512] pre-rope projection
                    (carries SX*SW scale; tables descale it)."""
                    ssl = bass.ts(s, 512)
                    if h < HPC:
                        dst = qt_4d[h // 2][s][:, :, h % 2, :]
                    else:
                        dst = kts[s][:]
                    raw = rtp.tile([128, 512], F32, tag="raw",
                                   name=f"raw{s}_{h}")
                    nc.scalar.copy(raw[:], src[:])
                    t1 = rtp.tile([128, 512], F32, tag="t1",
                                  name=f"t1_{s}_{h}")
                    nc.gpsimd.tensor_mul(t1[:], raw[:], cos_sb[:, ssl])
                    # rotate_half: walrus requires TT operands to share a
                    # start partition, so swap halves via copies first
                    # (partition-shifted copies are legal; signs live in sin_s)
                    sw = rtp.tile([128, 512], F32, tag="sw",
                                  name=f"sw{s}_{h}")
                    nc.vector.tensor_scalar_mul(sw[0:64, :],
                                                raw[64:128, :], 1.0)
                    nc.vector.tensor_scalar_mul(sw[64:128, :],
                                                raw[0:64, :], 1.0)
                    t2 = rtp.tile([128, 512], F32, tag="t2",
                                  name=f"t2_{s}_{h}")
                    nc.vector.tensor_mul(t2[:], sw[:], sin_sb[:, ssl])
                    if h < HPC:
                        t1v = t1.rearrange("p (i c) -> p i c", c=128)
                        t2v = t2.rearrange("p (i c) -> p i c", c=128)
                    else:
                        t1v, t2v = t1[:], t2[:]
                    nc.vector.tensor_add(dst, t1v, t2v)

                def v_pipeline(s, ps_v):
                    """ps_v: vT strip PSUM -> 4 v_aug tiles [k, dim].
                    Applies the fp8 descale (and the SA pre-scale)."""
                    vts = rtp.tile([128, 512], BF16, tag="vts", name=f"vts{s}")
                    nc.vector.tensor_scalar_mul(vts[:], ps_v[:],
                                                float(DESCALE_V))
                    vtp = upp.tile([128, 512], BF16, tag="util", name=f"vtp{s}")
                    for tt in range(4):
                        tsl = bass.ts(tt, 128)
                        nc.tensor.transpose(vtp[:, tsl], vts[:, tsl],
                                            ident_sb[:])
                        nc.vector.tensor_copy(vaug_sb[4 * s + tt][:, 0:D],
                                              vtp[:, tsl])

                def proj_round(s, heads, preamble=None, postamble=None):
                    """One k-loop computing fp8 3-term projections `heads`
                    (0..3 = q, 4 = k, 5 = v) for strip s into len(heads)
                    PSUM banks via DoubleRow matmuls. Base-weight terms are
                    emitted before residual-weight terms within each group
                    so group 0's split weight DMA doesn't stall the PE."""
                    ps = [ppp.tile([128, 512], F32, tag="proj",
                                   name=f"ps{s}_{h}") for h in heads]
                    for g in range(NKT // 4):
                        if preamble is not None:
                            preamble(g)
                        # one DMA carries x8 + rx8 for 4 hidden k-tiles
                        # (the kernel's very first ht DMA in halves aligned
                        # with the kpair passes, for a faster start)
                        ht_t = htp.tile([128, 2, 4, 512], E4, tag="ht",
                                        name=f"ht{s}_{g}_{heads[0]}")
                        if s == 0 and g == 0:
                            nc.sync.dma_start(ht_t[:, :, 0:2],
                                              ht_d[s, g][:, :, 0:2])
                            nc.sync.dma_start(ht_t[:, :, 2:4],
                                              ht_d[s, g][:, :, 2:4])
                        else:
                            nc.sync.dma_start(ht_t[:], ht_d[s, g])
                        if postamble is not None:
                            postamble(g)
                        # group 0 of strip 0: base-weight terms first, so the
                        # PE starts on the split weight DMA's first half
                        passes = ([(0, False), (1, False), (None, True)]
                                  if s == 0 and g == 0
                                  else [(0, True), (1, True)])
                        for kp0, with_resid in passes:
                            for kp in ([kp0] if kp0 is not None else [0, 1]):
                                gk = 2 * g + kp
                                first_k = gk == 0
                                last_k = gk == NKT // 2 - 1
                                for ps_t, h in zip(ps, heads):
                                    w8 = w_all[:, g, h, kp]
                                    rw8 = w_all[:, g, h + 6, kp]
                                    for th in range(2):  # token half
                                        osl = ps_t[:, bass.ts(th, 256)]
                                        rx = ht_t[:, 0, 2 * kp : 2 * kp + 2,
                                                  bass.ts(th, 256)]
                                        rr = ht_t[:, 1, 2 * kp : 2 * kp + 2,
                                                  bass.ts(th, 256)]
                                        if kp0 is not None:
                                            nc.tensor.matmul(
                                                osl, w8, rx,
                                                start=(first_k and th == 0),
                                                stop=False, perf_mode=DR)
                                            nc.tensor.matmul(
                                                osl, w8, rr,
                                                start=False, stop=False,
                                                perf_mode=DR)
                                        if with_resid:
                                            nc.tensor.matmul(
                                                osl, rw8, rx, start=False,
                                                stop=(last_k and th == 1),
                                                perf_mode=DR)
                    return ps

                def strip0_preamble(g):
                    # one 6KB-row DMA carries every tensor's chunk for group g
                    # (group 0 in halves: the first matmuls need only the
                    # base-weight half, so the PE starts ~2us sooner)
                    if g == 0:
                        nc.sync.dma_start(w_all[:, 0, 0:6], wall_d[0, :, 0:6])
                    else:
                        nc.sync.dma_start(w_all[:, g], wall_d[g])

                def strip0_postamble(g):
                    if g == 0:
                        nc.sync.dma_start(w_all[:, 0, 6:12],
                                          wall_d[0, :, 6:12])

                def table_chunk(s):
                    # rope-table chunk for strip s, just before its RoPE
                    csl = bass.ts(s, 512)
                    nc.sync.dma_start(cos_sb[:, csl], cos_d[:, csl])
                    nc.sync.dma_start(sin_sb[:, csl], sin_d[:, csl])
                    if s == 0:
                        nc.sync.dma_start(ident_sb[:], ident_d[:])
                    if s == 1:
                        nc.sync.dma_start(mask_sb[:], masks_d)

                def wo_chunk(s):
                    # wo is only needed in phase 2; trickle one of the four
                    # [128, 2, HID] e4m3 tiles per strip
                    r, hp = divmod(s, 2)
                    nc.sync.dma_start(wo_sb[r][hp][:], wo_d[r, hp])

                for s in range(NSTRIP - 1):
                    ps = proj_round(s, [0, 1, 2, 3, 4, 5],
                                    preamble=strip0_preamble if s == 0 else None,
                                    postamble=strip0_postamble if s == 0 else None)
                    table_chunk(s)
                    if s >= 1:
                        wo_chunk(s - 1)
                    v_pipeline(s, ps[5])
                    for h in range(HPC + 1):
                        rope(s, h, ps[h])

                # Last strip in two 3-bank rounds (hT re-streamed): round A's
                # banks drain during round B's matmuls, so phase 2's PSUM
                # pools don't stall on the phase-1 epilogue.
                s = NSTRIP - 1
                ps_a = proj_round(s, [0, 1, 4])
                table_chunk(s)
                wo_chunk(s - 1)
                for h in (0, 1, 4):
                    rope(s, h, ps_a[(0, 1, 4).index(h)])
                ps_b = proj_round(s, [5, 2, 3])
                wo_chunk(s)
                v_pipeline(s, ps_b[0])
                for h in (2, 3):
                    rope(s, h, ps_b[(5, 2, 3).index(h)])

                # hoist the first tiles' score blocks here: their matmuls
                # keep the PE fed while round B's rope chains drain and the
                # phase-2 pools wait on the phase-1 pool release
                for ei in (0, 1):
                    for ehp in (0, 1):
                        early_ses[(ei, ehp)] = score_block_gen(
                            ei, ehp,
                            ps_alloc=lambda nm: upp.tile(
                                [128, 512], F32, tag="util", name=nm),
                            se_alloc=lambda nm: pp.tile(
                                [128, 512], BF16, name=nm))

            # ---------------- phase 2: attention + out proj --------------
            with (
                tc.tile_pool(name="attn_sbuf", bufs=8) as asp,
                tc.tile_pool(name="attn_small", bufs=4) as asmall,
                tc.tile_pool(name="score_psum", bufs=3, space="PSUM") as spp,
                tc.tile_pool(name="oaug_psum", bufs=2, space="PSUM") as opp,
                tc.tile_pool(name="oproj_psum", bufs=3, space="PSUM") as prp,
            ):

                def oproj(i, at8_list, rat8_list, otile, ns_range):
                    """fp8 3-term out-proj for token tile i, out-col slices
                    ns_range. at8/rat8: per head-pair [128, 2, 128] e4m3
                    (x SA)."""
                    isl = bass.ts(i, 128)
                    for ns in ns_range:
                        po = prp.tile([128, 512], F32, tag="oproj",
                                      name=f"po{i}_{ns}")
                        n_in = 12
                        n = 0
                        for hp in range(2):
                            for th in range(2):
                                osl = po[:, bass.ts(th, 256)]
                                csl = bass.ds(512 * ns + 256 * th, 256)
                                w8 = wo_sb[0][hp][:, :, csl]
                                rw8 = wo_sb[1][hp][:, :, csl]
                                a8 = at8_list[hp][:]
                                ra8 = rat8_list[hp][:]
                                for lhs, rhs in ((a8, w8), (ra8, w8),
                                                 (a8, rw8)):
                                    nc.tensor.matmul(
                                        osl, lhs, rhs,
                                        start=(n == 0), stop=(n == n_in - 1),
                                        perf_mode=DR)
                                    n += 1
                        osb = otile[:, bass.ts(ns, 512)]
                        if ns % 2 == 0:
                            nc.gpsimd.tensor_scalar_mul(osb, po[:],
                                                        float(DESCALE_O))
                        else:
                            nc.scalar.mul(osb, po[:], float(DESCALE_O))
                        if i == 0:
                            # last-processed tile: drain progressively so the
                            # epilogue isn't one long serial copy+DMA chain
                            if ns % 2 == 1:
                                csl = bass.ds(512 * (ns - 1), 1024)
                                nc.sync.dma_start(out_d[isl, csl],
                                                  otile[:, csl])
                        else:
                            if ns == NOUT // 2 - 1:
                                nc.sync.dma_start(out_d[isl, 0 : HID // 2],
                                                  otile[:, 0 : HID // 2])
                            if ns == NOUT - 1:
                                nc.sync.dma_start(
                                    out_d[isl, HID // 2 : HID],
                                    otile[:, HID // 2 : HID])

                def score_block(i, hp):
                    if (i, hp) in early_ses:
                        return early_ses[(i, hp)]
                    return score_block_gen(
                        i, hp,
                        ps_alloc=lambda nm: spp.tile(
                            [128, 512], F32, tag="score", name=nm),
                        se_alloc=lambda nm: asp.tile(
                            [128, 512], BF16, tag="sexp", name=nm))

                def pv_block(i, hp, ses, at8_pair, rat8_pair):
                    """PV matmuls + normalize + transpose + fp8 quantize."""
                    jl = jobs[i]
                    njobs = len(jl)
                    # one bank holds both heads' [q, d+1] PV outputs
                    ps_o = opp.tile([128, 2, D + 1], F32, tag="oaug",
                                    name=f"pso{i}_{hp}")
                    for p0 in range(0, njobs, 2):
                        se = ses[p0 // 2]
                        for q, (j, mid) in enumerate(jl[p0 : p0 + 2]):
                            jj = p0 + q
                            for m in range(2):
                                nc.tensor.matmul(
                                    ps_o[:, m],
                                    se[:, bass.ds(256 * q + 128 * m, 128)],
                                    vaug_sb[j][:],
                                    start=(jj == 0 and m == 0),
                                    stop=(jj == njobs - 1 and m == 1))
                    for m in range(2):
                        h = 2 * hp + m
                        recip = asmall.tile([128, 1], F32, tag="recip",
                                            name=f"rc{i}_{h}")
                        nc.vector.reciprocal(recip[:],
                                             ps_o[:, m, D : D + 1])
                        a_n = asp.tile([128, 128], BF16, tag="anorm",
                                       name=f"an{i}_{h}")
                        nc.vector.tensor_scalar_mul(a_n[:],
                                                    ps_o[:, m, 0:D],
                                                    recip[:])
                        at_p = spp.tile([128, 128], BF16, tag="score",
                                        name=f"atp{i}_{h}")
                        nc.tensor.transpose(at_p[:], a_n[:], ident_sb[:])
                        # quantize to e4m3 + residual (values carry SA);
                        # DVE so the at_p score bank drains fast
                        nc.vector.tensor_copy(at8_pair[:, m], at_p[:])
                        nc.vector.tensor_sub(rat8_pair[:, m], at_p[:],
                                             at8_pair[:, m])

                # tile 0 (single job, shortest serial chain) goes last so
                # the end-of-kernel exp->pv->quantize->oproj tail is minimal
                order = list(range(1, NT)) + [0]
                prev_i, prev_at = None, None
                for i in order:
                    at8_pair = [asp.tile([128, 2, 128], E4, tag="at8",
                                         bufs=5, name=f"at8_{i}_{hp}")
                                for hp in range(2)]
                    rat8_pair = [asp.tile([128, 2, 128], E4, tag="rat8",
                                          bufs=5, name=f"rat8_{i}_{hp}")
                                 for hp in range(2)]
                    if prev_at is not None:
                        po_tile = asp.tile([128, HID], BF16, tag="obat",
                                           bufs=2, name=f"ob{prev_i}")
                    # emission order keeps the in-order PE queue fed: the
                    # score blocks run while earlier exps drain, and the
                    # lagged out-proj chunks fill the exp/mask latency
                    ses0 = score_block(i, 0)
                    if prev_at is not None:
                        oproj(prev_i, *prev_at, po_tile, range(0, NOUT // 2))
                    ses1 = score_block(i, 1)
                    pv_block(i, 0, ses0, at8_pair[0], rat8_pair[0])
                    if prev_at is not None:
                        oproj(prev_i, *prev_at, po_tile, range(NOUT // 2, NOUT))
                    pv_block(i, 1, ses1, at8_pair[1], rat8_pair[1])
                    prev_i, prev_at = i, (at8_pair, rat8_pair)
                ot = asp.tile([128, HID], BF16, tag="obat", bufs=2,
                              name=f"ob{prev_i}")
                oproj(prev_i, *prev_at, ot, range(NOUT))

    nc.compile()
    return nc


def _get_nc(cu_seqlens):
    key = np.asarray(cu_seqlens).tobytes()
    if key not in _cache:
        jobs, masks_np, cos_t, sin_s, ident = _host_prep(cu_seqlens)
        nc = _build(jobs, masks_np.shape[1] // 256)
        _cache[key] = (nc, masks_np, cos_t, sin_s, ident)
    return _cache[key]


def kernel(hidden_states, Wq, Wk, Wv, Wo, cu_seqlens):
    hidden_states = np.asarray(hidden_states)
    Wq, Wk, Wv, Wo = (np.asarray(a) for a in (Wq, Wk, Wv, Wo))
    cu_seqlens = np.asarray(cu_seqlens)
    nc, masks_np, cos_t, sin_s, ident = _get_nc(cu_seqlens)

    # hT in fp8: base + residual at scale SX
    ht = np.ascontiguousarray(hidden_states.T) * np.float32(SX)
    h8, hr8 = _q8(ht)
    # tile for contiguous DMA: [NSTRIP, NKT//4, 128, 2, 4, 512] — each
    # DMA carries 4 hidden k-tiles of x8 then the matching rx8
    def tile_ht(a):
        return np.ascontiguousarray(
            a.reshape(NKT // 4, 4, 128, NSTRIP, 512).transpose(3, 0, 2, 1, 4)
        ).reshape(NSTRIP, NKT // 4, 128, 1, 4, 512)
    ht_tiled = np.concatenate([tile_ht(h8), tile_ht(hr8)], axis=3)

    in_maps = []
    for c in range(N_CORES):
        # flat per-tensor lhsT layout: [128, HID] with col = 128*ktile + m
        def tile_wq(a):
            return np.ascontiguousarray(
                a.reshape(NKT, 128, HPC, 128).transpose(2, 1, 0, 3)
            ).reshape(HPC, 128, HID)
        def tile_wkv(a):
            return np.ascontiguousarray(
                a.reshape(NKT, 128, 128).transpose(1, 0, 2)).reshape(128, HID)
        wq_c = Wq[:, QD * c : QD * (c + 1)].astype(np.float32) * np.float32(SW)
        wq8, wqr = _q8(wq_c)
        wk8, wkr = _q8(Wk[:, D * c : D * (c + 1)].astype(np.float32)
                       * np.float32(SW))
        wv8, wvr = _q8(Wv[:, D * c : D * (c + 1)].astype(np.float32)
                       * np.float32(SW))
        # pack the 12 tensors group-interleaved:
        # wall[g, :, t, :] = flat_t[:, 512g : 512(g+1)]
        flats = ([tile_wq(wq8)[h] for h in range(HPC)]
                 + [tile_wkv(wk8), tile_wkv(wv8)]
                 + [tile_wq(wqr)[h] for h in range(HPC)]
                 + [tile_wkv(wkr), tile_wkv(wvr)])
        stackw = np.stack(flats)  # [12, 128, HID]
        wall = np.ascontiguousarray(
            stackw.reshape(12, 128, NKT // 4, 512).transpose(2, 1, 0, 3)
        ).reshape(NKT // 4, 128, 12, 2, 2, 128)
        # wo: [2, hpair, 128 d, 2 head-in-pair, HID]
        wo_c = Wo[QD * c : QD * (c + 1), :].astype(np.float32) \
            * np.float32(SWO)
        wo8, wor = _q8(wo_c)
        def tile_wo(a):
            return np.ascontiguousarray(
                a.reshape(2, 2, 128, HID).transpose(0, 2, 1, 3))
        wo_t = np.stack([tile_wo(wo8), tile_wo(wor)])
        in_maps.append({
            "ht": ht_tiled, "wall": wall, "wo": wo_t,
            "cos_t": cos_t, "sin_s": sin_s, "ident": ident,
            "masks": masks_np,
        })

    res = bass_utils.run_bass_kernel_spmd(nc, in_maps,
                                          core_ids=list(range(N_CORES)))
    out = res.results[0]["out"].astype(np.float64)
    for c in range(1, N_CORES):
        out += res.results[c]["out"].astype(np.float64)
    return out.astype(np.float32)
